# revision 12
# baseline (speedup 1.0000x reference)
"""Trainium2 Bass kernel for nn_CrossEntropyLossWeight3.

Math: per row b of predict/target [B,16]:
  probs   = softmax(predict[b])
  pre     = argmax(predict[b]);  tar = argmax(target[b])
  w       = 0 if pre==tar else penalty[tar, pre]
  loss_b  = w * probs[pre]
out = mean_b(loss_b)

Key identities used on-device:
  probs[pre]   = exp(max(x)) / sum(exp(x))      (softmax at its own argmax)
  penalty[i,j] = max(c_i,c_j)/(c_i+c_j) with distinct per-class counts c;
  with u = c[pre], v = c[tar]:  w = (u != v) * max(u,v)/(u+v).
  counts/1000 (9 bits, exact) are embedded into the low mantissa bits of the
  raw inputs, so one fused embed+segmented-max DVE scan per tensor yields
  the row max together with its argmax's class count (<= 2^-14 relative
  perturbation). Two more fused custom DVE ops evaluate the whole per-row
  weight formula straight from the embedded maxima:
    WNUM = (u!=v) * max(u,v)        SPD = u + v
  so loss_b = WNUM * exp(m) / (SPD * sumexp).

v6 engine balance (per [128, 256*16] tile; single sync HWDGE ring streams
both tensors at a measured ~428 GB/s => ~9.4us/tile of DMA):
  - DVE     : two embed+segmax f32 scans (2 x 4.4us) + WNUM/SPD/recip per
              tile (~1.3us)  => ~10.1us/tile, the critical engine
  - ACT     : exp(predict) f32->bf16 (3.7us) + exp(m) (~0.3us)
  - TensorE : row sums of E as 16 PSUM-accumulated matmuls with identity
              weights (rhs = E[:, :, w], w=0..15) -> s[p,r] lands in PSUM
              in f32, ~2-3us/tile on an otherwise idle engine
  - GPSIMD  : only the small per-tile formula mults den/num/num2/acc
              (Q7 is ~2x slower under full DMA load; it gets no streaming
              work at all)
  - DMA     : both input streams + out on the SP (sync) ring so ACT's exp
              never sits in front of a dma_start issue
  - formula : per tile, split into F1 (wn/sp/em/den/num, emitted with the
              tile) and F2 (rec/num2/acc, deferred one tile) so no engine
              head-of-line stalls on a cross-engine dependency
Sharding: pure data parallel over 8 cores (batch split); each core returns
per-partition partial sums [128,256]; host reduces and divides by B.
"""

import sys

sys.path.insert(0, "/opt/trn_rl_repo")

import numpy as np

import concourse.bass as bass
import concourse.bacc as bacc
import concourse.tile as tile
from concourse import mybir
from concourse.bass_utils import run_bass_kernel_spmd

B, W = 2097152, 16
NCORES = 8
BS = B // NCORES          # rows per core
P = 128                   # SBUF partitions
R = 256                   # rows per partition per tile
F = R * W                 # free elems per partition per tile
TILE_ROWS = P * R
NT = BS // TILE_ROWS      # tiles per core

LABELS_NUM_COUNT = [500000, 120000, 80000, 45000, 30000, 250000, 15000, 9000,
                    60000, 7000, 180000, 22000, 11000, 95000, 5000, 40000]

f32 = mybir.dt.float32
bf16 = mybir.dt.bfloat16
u32 = mybir.dt.uint32
AX = mybir.AxisListType
OP = mybir.AluOpType
ACT = mybir.ActivationFunctionType

PAYLOAD_BITS = 9          # counts/1000 <= 500 fits in 9 bits exactly
PAYLOAD_MASK = (1 << PAYLOAD_BITS) - 1
F_2P23 = 8388608.0        # bit pattern 0x4B000000; OR'ing these bits onto the
                          # 9-bit payload makes the exact float 2^23 + payload
F_2P24 = 16777216.0


def _register_custom_ops():
    """Three runtime-registered custom DVE ops.

    EMBMAX_SEG_ANT: fused "embed payload + segmented max" scan (see v2/v3
      history): body = Scan(MAX, ((x|c)^c)|pay, _subdim_step=Zero) over a
      [P, S, 16] view; stride-0 out leaves per-segment maxima in [P, S].
      The OR/XOR form avoids an AND with 0xFFFFFE00 (NaN bit pattern).
    WNUM_ANT(me, mt; s0=mask, s1=2^23): with u' = (me & mask) | bits(s1),
      v' = (mt & mask) | bits(s1)  (both exact floats 2^23 + count):
      out = (u' != v') * (max(u',v') - 2^23) = (u!=v)*max(u,v).
    SPD_ANT(me, mt; s0, s1, imm2=2^24): out = u' + v' - 2^24 = u + v.
    """
    import numpy as np_

    from concourse.dve_spec import (
        Spec, Src0, Src1, C0, C1, C2, Bin, AluOp, lower, ne, maxx, Zero,
    )
    from concourse.dve_ops import (
        DveOp,
        OPS,
        CUSTOM_DVE_SPECS,
        _SUB_OPCODE_FOR_NAME,
        _CUSTOM_DVE_ROW_BASE,
        _COMPILE_CACHE,
    )
    from concourse.dve_uop import DveOpSpec
    import concourse.dve_spec as ds

    def reg(name, spec, rd1):
        for o in OPS:
            if o.name == name:
                return o
        shas = {}
        for ver in ("v3", "v4"):
            uops = lower(spec, ver=ver)
            s = DveOpSpec(
                name=name,
                opcode=_CUSTOM_DVE_ROW_BASE + len(OPS),
                uops=uops,
                rd1_en=rd1,
            )
            shas[ver] = s.sha(ver)
        op = DveOp(name, spec, subdim=False, uops_sha=shas)
        _SUB_OPCODE_FOR_NAME[name] = _CUSTOM_DVE_ROW_BASE + len(OPS)
        OPS.append(op)
        CUSTOM_DVE_SPECS[name] = spec
        return op

    embed_expr = Bin(
        AluOp.BITWISE_OR,
        Bin(AluOp.BITWISE_XOR, Bin(AluOp.BITWISE_OR, Src0, C0), C0),
        Src1,
    )

    def _ref_embmax(in0, in1, s0, s1, imm2):
        emb = (
            ((in0.view(np_.uint32) | PAYLOAD_MASK) ^ PAYLOAD_MASK)
            | in1.view(np_.uint32)
        ).view(np_.float32)
        return np_.maximum.accumulate(emb, axis=-1)

    def reg_embmax():
        name = "EMBMAX_SEG_ANT"
        for o in OPS:
            if o.name == name:
                return o
        seg = ds.Scan(op=AluOp.MAX, expr=embed_expr, init=None, _subdim_step=Zero)
        spec = Spec(body=seg, reference=_ref_embmax)
        orig_so, orig_nas = ds._scan_overrides, ds._node_as_stage

        def patched_so(scans, node_stage):
            seed, step = {}, {}
            for scan in scans:
                d = node_stage[scan]
                init = (
                    scan.init
                    if scan.init is not None
                    else ds._ACCUM_IDENTITY[scan.op]
                )
                seed[d] = orig_nas(init)
                if scan._subdim_step is not None:
                    step[d] = ds._Stage(AluOp.BYPASS, scan.expr)
            return seed, step

        def patched_nas(e):
            if isinstance(e, ds.Scan) and e._subdim_step is not None:
                return ds._Stage(e.op, ds.AluInp.CURR_ALU_OUT, e.expr)
            return orig_nas(e)

        uops_by_ver, shas = {}, {}
        ds._scan_overrides, ds._node_as_stage = patched_so, patched_nas
        try:
            for ver in ("v3", "v4"):
                uops_by_ver[ver] = lower(spec, ver=ver)
        finally:
            ds._scan_overrides, ds._node_as_stage = orig_so, orig_nas
        opcode = _CUSTOM_DVE_ROW_BASE + len(OPS)
        for ver in ("v3", "v4"):
            s = DveOpSpec(name=name, opcode=opcode, uops=uops_by_ver[ver], rd1_en=True)
            shas[ver] = s.sha(ver)
            _COMPILE_CACHE[(name, ver)] = s
        op = DveOp(name, spec, subdim=True, uops_sha=shas)
        _SUB_OPCODE_FOR_NAME[name] = opcode
        OPS.append(op)
        CUSTOM_DVE_SPECS[name] = spec
        return op

    def _uprime(src):
        return Bin(AluOp.BITWISE_OR, Bin(AluOp.BITWISE_AND, src, C0), C1)

    def _np_uprime(x):
        return (
            (x.view(np_.uint32) & PAYLOAD_MASK) | np_.uint32(0x4B000000)
        ).view(np_.float32)

    up_e, vp_e = _uprime(Src0), _uprime(Src1)
    wnum_spec = Spec(
        body=Bin(
            AluOp.MULTIPLY,
            ne(up_e, vp_e),
            Bin(AluOp.SUBTRACT, maxx(up_e, vp_e), C1),
        ),
        reference=lambda in0, in1, s0, s1, imm2: np_.where(
            _np_uprime(in0) != _np_uprime(in1),
            np_.maximum(_np_uprime(in0), _np_uprime(in1)) - np_.float32(F_2P23),
            np_.float32(0.0),
        ).astype(np_.float32),
    )
    spd_spec = Spec(
        body=Bin(
            AluOp.SUBTRACT, Bin(AluOp.ADD, up_e, vp_e), C2
        ),
        reference=lambda in0, in1, s0, s1, imm2: (
            _np_uprime(in0) + _np_uprime(in1) - np_.float32(F_2P24)
        ).astype(np_.float32),
    )

    embed = reg_embmax()
    wnum = reg("WNUM_ANT", wnum_spec, rd1=True)
    spd = reg("SPD_ANT", spd_spec, rd1=True)
    return embed, wnum, spd


TPB = 4                    # tiles per formula block
BW = TPB * R               # formula block width (1024)
NBLK = NT // TPB           # formula blocks per core


def _emit_tile(nc, pools, pred_v, targ_v, pay_b, ident_b, t, embed_op,
               mask_ap, me, mt, e2, s2a, s2b):
    """Streaming part for one [128, R*16] tile. Row stats land in column
    slice t%TPB of the block stats tiles me/mt; exp lands (w-major, so
    matmul rhs slices are contiguous) in half t%2 of the pair tile e2; on
    odd tiles 16 paired matmuls produce row sums of both halves into the
    alternating PSUM tiles s2a/s2b."""
    io_pool = pools[0]
    cols = slice((t % TPB) * R, (t % TPB + 1) * R)

    # both input streams on the sync HWDGE ring: SP issues nothing else, so
    # dma_starts go out back-to-back and are never stuck behind an ACT op
    xp = io_pool.tile([P, F], f32, tag="xp")
    nc.sync.dma_start(out=xp[:, :], in_=pred_v[t])
    xt = io_pool.tile([P, F], f32, tag="xt")
    nc.sync.dma_start(out=xt[:, :], in_=targ_v[t])

    # fused embed + segmented max over RAW predict on DVE; runs concurrently
    # with the exp pass on ACT (both only read xp)
    xp3 = xp[:, :].rearrange("p (r w) -> p r w", w=W)
    nc.vector._custom_dve(
        embed_op,
        out=me[:, cols].unsqueeze(2).broadcast_to([P, R, W]),
        in0=xp3, in1=pay_b, s0=mask_ap,
    )

    # E = exp(predict) on ScalarE into half t%2 of the bf16 pair tile,
    # TRANSPOSED within the partition to w-major [p, (w, r2)] so that each
    # matmul below reads a CONTIGUOUS [128, 2R] slice (a strided rhs read
    # ran at ~1.8ns/col; contiguous should stream near 1 col/cycle)
    e2t = e2[:, :].rearrange("p (w hr) -> p hr w", w=W)
    nc.scalar.activation(e2t[:, (t % 2) * R:(t % 2 + 1) * R, :], xp3,
                         ACT.Exp)

    # target side: fused embed + segmented max on DVE
    xt3 = xt[:, :].rearrange("p (r w) -> p r w", w=W)
    nc.vector._custom_dve(
        embed_op,
        out=mt[:, cols].unsqueeze(2).broadcast_to([P, R, W]),
        in0=xt3, in1=pay_b, s0=mask_ap,
    )

    # row sums of E on the (otherwise idle) TensorE: 16 matmuls per tile
    # PAIR with identity weights, one per class column, PSUM-accumulated in
    # f32 with even/odd w on separate PSUM banks so consecutive matmuls
    # never RMW-chain on the same bank:
    #   s2a+s2b = sum_w I.T @ E2t[:, w, :]  ->  [128, 2R] each in PSUM
    if t % 2 == 1:
        for w in range(W):
            dst = s2a if w % 2 == 0 else s2b
            nc.tensor.matmul(
                out=dst[:, :], lhsT=ident_b[:, :],
                rhs=e2[:, w * 2 * R:(w + 1) * 2 * R],
                start=(w < 2), stop=(w >= W - 2),
            )


def _emit_f1(nc, pools, me, mt, s2c, ops, mask_ap, last):
    """Formula stage 1 for one [128, BW] block (emitted with its last
    tile): everything that only needs me/mt/s.
      wn = (u!=v)*max(u,v)   sp = u+v   em = exp(m)
      den = sp * sumexp      num = wn * em
    den/num run on GPSIMD (idle) except for the last block, where DVE's
    ~1.2us ops shorten the post-DMA tail. Returns (den, num)."""
    fp_pool = pools[3]
    _, wnum_op, spd_op = ops
    mul_eng = nc.vector if last else nc.gpsimd

    wn = fp_pool.tile([P, BW], f32, tag="wn")
    nc.vector._custom_dve(wnum_op, out=wn[:, :], in0=me[:, :], in1=mt[:, :],
                          s0=mask_ap, s1=F_2P23)
    sp = fp_pool.tile([P, BW], f32, tag="sp")
    nc.vector._custom_dve(spd_op, out=sp[:, :], in0=me[:, :], in1=mt[:, :],
                          s0=mask_ap, s1=F_2P23, imm2=F_2P24)
    # em = exp(m): payload bits perturb m by <= 2^-14 relative — in budget
    em = fp_pool.tile([P, BW], f32, tag="em")
    nc.scalar.activation(em[:, :], me[:, :], ACT.Exp)

    den = fp_pool.tile([P, BW], f32, tag="dn")
    mul_eng.tensor_tensor(den[:, :], sp[:, :], s2c[:, :], op=OP.mult)
    num = fp_pool.tile([P, BW], f32, tag="nm")
    mul_eng.tensor_tensor(num[:, :], wn[:, :], em[:, :], op=OP.mult)
    return den, num


def _emit_f2(nc, pools, res_sl, den, num, last):
    """Formula stage 2 (emitted one tile later so no engine head-of-line
    stalls on a cross-engine dep): res_block = num / den (host sums res)."""
    fp_pool = pools[3]
    rec = fp_pool.tile([P, BW], f32, tag="rc")
    nc.vector.reciprocal_approx_fast(out=rec[:, :], in_=den[:, :])
    mul_eng = nc.vector if last else nc.gpsimd
    mul_eng.tensor_tensor(res_sl, num[:, :], rec[:, :], op=OP.mult)


def _emit_pass(nc, pools, pred_v, targ_v, pay_b, ident_b, res, ops, mask_ap):
    _, work_pool, stats_pool, fp_pool, ps_pool = pools
    embed_op = ops[0]
    pend = None            # (den, num) of the previous block
    for k in range(NBLK):
        me = stats_pool.tile([P, BW], f32, tag="me")
        mt = stats_pool.tile([P, BW], f32, tag="mt")
        s2c = fp_pool.tile([P, BW], f32, tag="s2c")
        for half in range(TPB // 2):
            e2 = work_pool.tile([P, 2 * F], bf16, tag="e2")
            s2a = ps_pool.tile([P, 2 * R], f32, tag="s2a")
            s2b = ps_pool.tile([P, 2 * R], f32, tag="s2b")
            for sub in range(2):
                t = k * TPB + 2 * half + sub
                _emit_tile(nc, pools, pred_v, targ_v, pay_b, ident_b, t,
                           embed_op, mask_ap, me, mt, e2, s2a, s2b)
                # interleave the previous block's F2 early in this block
                if pend is not None and half == 0 and sub == 1:
                    _emit_f2(nc, pools, res[:, (k - 1) * BW:k * BW], *pend,
                             last=False)
                    pend = None
            # merge the two PSUM halves into the block s2c tile: only one
            # PSUM operand is allowed per instruction, so ACT drains s2a
            # to SBUF and DVE adds s2b on top
            sl = slice(half * 2 * R, (half + 1) * 2 * R)
            nc.scalar.activation(s2c[:, sl], s2a[:, :], ACT.Copy)
            nc.vector.tensor_tensor(s2c[:, sl], s2c[:, sl], s2b[:, :],
                                    op=OP.add)
        pend = _emit_f1(nc, pools, me, mt, s2c, ops, mask_ap,
                        last=(k == NBLK - 1))
    _emit_f2(nc, pools, res[:, (NBLK - 1) * BW:], *pend, last=True)


def _build_program():
    nc = bacc.Bacc("TRN2", target_bir_lowering=False, debug=False)
    pred = nc.dram_tensor("predict", [BS, W], f32, kind="ExternalInput")
    targ = nc.dram_tensor("target", [BS, W], f32, kind="ExternalInput")
    pay = nc.dram_tensor("payload", [P, W], u32, kind="ExternalInput")
    ident = nc.dram_tensor("ident", [P, P], f32, kind="ExternalInput")
    out = nc.dram_tensor("out", [P, NBLK * BW], f32, kind="ExternalOutput")

    pred_v = pred[:, :].rearrange("(t p r) w -> t p (r w)", t=NT, p=P, r=R)
    targ_v = targ[:, :].rearrange("(t p r) w -> t p (r w)", t=NT, p=P, r=R)

    with tile.TileContext(nc) as tc:
        with (
            tc.tile_pool(name="io", bufs=3) as io_pool,
            tc.tile_pool(name="work", bufs=2) as work_pool,
            tc.tile_pool(name="stats", bufs=2) as stats_pool,
            tc.tile_pool(name="fp", bufs=1) as fp_pool,
            tc.psum_pool(name="ps", bufs=3) as ps_pool,
            tc.tile_pool(name="const", bufs=1) as const_pool,
        ):
            pay_t = const_pool.tile([P, W], u32, tag="pay")
            nc.gpsimd.dma_start(out=pay_t[:, :], in_=pay[:, :])
            pay_b = pay_t[:, :].unsqueeze(1).broadcast_to([P, R, W]).bitcast(f32)

            ident_t = const_pool.tile([P, P], f32, tag="idf")
            nc.gpsimd.dma_start(out=ident_t[:, :], in_=ident[:, :])
            ident_b = const_pool.tile([P, P], bf16, tag="idb")
            nc.scalar.activation(ident_b[:, :], ident_t[:, :], ACT.Copy)

            mask_t = const_pool.tile([P, 1], u32, tag="mask")
            nc.vector.memset(mask_t[:, :], PAYLOAD_MASK)
            mask_ap = mask_t[:, :1].bitcast(f32)

            res = const_pool.tile([P, NBLK * BW], f32, tag="res")

            ops = _register_custom_ops()
            pools = (io_pool, work_pool, stats_pool, fp_pool, ps_pool)
            _emit_pass(nc, pools, pred_v, targ_v, pay_b, ident_b, res, ops,
                       mask_ap)

            nc.sync.dma_start(out=out[:, :], in_=res[:, :])
    nc.compile()
    return nc


_CACHE = {}


def _run(predict, target, trace=False):
    if "nc" not in _CACHE:
        _CACHE["nc"] = _build_program()
    nc = _CACHE["nc"]

    predict = np.ascontiguousarray(np.asarray(predict, dtype=np.float32))
    target = np.ascontiguousarray(np.asarray(target, dtype=np.float32))
    payload = np.broadcast_to(
        (np.asarray(LABELS_NUM_COUNT, dtype=np.uint32) // 1000)[None, :], (P, W)
    ).copy()
    ident = np.eye(P, dtype=np.float32)

    in_maps = []
    for i in range(NCORES):
        in_maps.append(
            {
                "predict": predict[i * BS : (i + 1) * BS],
                "target": target[i * BS : (i + 1) * BS],
                "payload": payload,
                "ident": ident,
            }
        )
    res = run_bass_kernel_spmd(nc, in_maps, core_ids=list(range(NCORES)), trace=trace)
    total = np.float64(0.0)
    for r in res.results:
        total += np.float64(r["out"].astype(np.float64).sum())
    value = np.float32(total / B)
    return np.asarray(value, dtype=np.float32), res


def kernel(predict, target, penalty_matrix=None):
    value, _ = _run(predict, target, trace=False)
    return value



# revision 14
# speedup vs baseline: 1.5885x; 1.5885x over previous
"""Trainium2 Bass kernel for nn_CrossEntropyLossWeight3.

Math: per row b of predict/target [B,16]:
  probs   = softmax(predict[b])
  pre     = argmax(predict[b]);  tar = argmax(target[b])
  w       = 0 if pre==tar else penalty[tar, pre]
  loss_b  = w * probs[pre]
out = mean_b(loss_b)

Key identities used on-device:
  probs[pre]   = exp(max(x)) / sum(exp(x))      (softmax at its own argmax)
  penalty[i,j] = max(c_i,c_j)/(c_i+c_j) with distinct per-class counts c;
  with u = c[pre], v = c[tar]:  w = (u != v) * max(u,v)/(u+v).
  counts/1000 (9 bits, exact) are embedded into the low mantissa bits of the
  raw inputs, so one fused embed+segmented-max DVE scan per tensor yields
  the row max together with its argmax's class count (<= 2^-14 relative
  perturbation). Two more fused custom DVE ops evaluate the whole per-row
  weight formula straight from the embedded maxima:
    WNUM = (u!=v) * max(u,v)        SPD = u + v
  so loss_b = WNUM * exp(m) / (SPD * sumexp).

v6 engine balance (per [128, 256*16] tile; single sync HWDGE ring streams
both tensors at a measured ~428 GB/s => ~9.4us/tile of DMA):
  - DVE     : two embed+segmax f32 scans (2 x 4.4us) + WNUM/SPD/recip per
              tile (~1.3us)  => ~10.1us/tile, the critical engine
  - ACT     : exp(predict) f32->bf16 (3.7us) + exp(m) (~0.3us)
  - TensorE : row sums of E as 16 PSUM-accumulated matmuls with identity
              weights (rhs = E[:, :, w], w=0..15) -> s[p,r] lands in PSUM
              in f32, ~2-3us/tile on an otherwise idle engine
  - GPSIMD  : only the small per-tile formula mults den/num/num2/acc
              (Q7 is ~2x slower under full DMA load; it gets no streaming
              work at all)
  - DMA     : both input streams + out on the SP (sync) ring so ACT's exp
              never sits in front of a dma_start issue
  - formula : per tile, split into F1 (wn/sp/em/den/num, emitted with the
              tile) and F2 (rec/num2/acc, deferred one tile) so no engine
              head-of-line stalls on a cross-engine dependency
Sharding: pure data parallel over 8 cores (batch split); each core returns
per-partition partial sums [128,256]; host reduces and divides by B.
"""

import sys

sys.path.insert(0, "/opt/trn_rl_repo")

import numpy as np

import concourse.bass as bass
import concourse.bacc as bacc
import concourse.tile as tile
from concourse import mybir
from concourse.bass_utils import run_bass_kernel_spmd

B, W = 2097152, 16
NCORES = 8
BS = B // NCORES          # rows per core
P = 128                   # SBUF partitions
R = 256                   # rows per partition per tile
F = R * W                 # free elems per partition per tile
TILE_ROWS = P * R
NT = BS // TILE_ROWS      # tiles per core

LABELS_NUM_COUNT = [500000, 120000, 80000, 45000, 30000, 250000, 15000, 9000,
                    60000, 7000, 180000, 22000, 11000, 95000, 5000, 40000]

f32 = mybir.dt.float32
bf16 = mybir.dt.bfloat16
u32 = mybir.dt.uint32
AX = mybir.AxisListType
OP = mybir.AluOpType
ACT = mybir.ActivationFunctionType

PAYLOAD_BITS = 9          # counts/1000 <= 500 fits in 9 bits exactly
PAYLOAD_MASK = (1 << PAYLOAD_BITS) - 1
F_2P23 = 8388608.0        # bit pattern 0x4B000000; OR'ing these bits onto the
                          # 9-bit payload makes the exact float 2^23 + payload
F_2P24 = 16777216.0


def _register_custom_ops():
    """Three runtime-registered custom DVE ops.

    EMBMAX_SEG_ANT: fused "embed payload + segmented max" scan (see v2/v3
      history): body = Scan(MAX, ((x|c)^c)|pay, _subdim_step=Zero) over a
      [P, S, 16] view; stride-0 out leaves per-segment maxima in [P, S].
      The OR/XOR form avoids an AND with 0xFFFFFE00 (NaN bit pattern).
    WNUM_ANT(me, mt; s0=mask, s1=2^23): with u' = (me & mask) | bits(s1),
      v' = (mt & mask) | bits(s1)  (both exact floats 2^23 + count):
      out = (u' != v') * (max(u',v') - 2^23) = (u!=v)*max(u,v).
    SPD_ANT(me, mt; s0, s1, imm2=2^24): out = u' + v' - 2^24 = u + v.
    """
    import numpy as np_

    from concourse.dve_spec import (
        Spec, Src0, Src1, C0, C1, C2, Bin, AluOp, lower, ne, maxx, Zero,
    )
    from concourse.dve_ops import (
        DveOp,
        OPS,
        CUSTOM_DVE_SPECS,
        _SUB_OPCODE_FOR_NAME,
        _CUSTOM_DVE_ROW_BASE,
        _COMPILE_CACHE,
    )
    from concourse.dve_uop import DveOpSpec
    import concourse.dve_spec as ds

    def reg(name, spec, rd1):
        for o in OPS:
            if o.name == name:
                return o
        shas = {}
        for ver in ("v3", "v4"):
            uops = lower(spec, ver=ver)
            s = DveOpSpec(
                name=name,
                opcode=_CUSTOM_DVE_ROW_BASE + len(OPS),
                uops=uops,
                rd1_en=rd1,
            )
            shas[ver] = s.sha(ver)
        op = DveOp(name, spec, subdim=False, uops_sha=shas)
        _SUB_OPCODE_FOR_NAME[name] = _CUSTOM_DVE_ROW_BASE + len(OPS)
        OPS.append(op)
        CUSTOM_DVE_SPECS[name] = spec
        return op

    embed_expr = Bin(
        AluOp.BITWISE_OR,
        Bin(AluOp.BITWISE_XOR, Bin(AluOp.BITWISE_OR, Src0, C0), C0),
        Src1,
    )

    def _ref_embmax(in0, in1, s0, s1, imm2):
        emb = (
            ((in0.view(np_.uint32) | PAYLOAD_MASK) ^ PAYLOAD_MASK)
            | in1.view(np_.uint32)
        ).view(np_.float32)
        return np_.maximum.accumulate(emb, axis=-1)

    def reg_embmax():
        name = "EMBMAX_SEG_ANT"
        for o in OPS:
            if o.name == name:
                return o
        seg = ds.Scan(op=AluOp.MAX, expr=embed_expr, init=None, _subdim_step=Zero)
        spec = Spec(body=seg, reference=_ref_embmax)
        orig_so, orig_nas = ds._scan_overrides, ds._node_as_stage

        def patched_so(scans, node_stage):
            seed, step = {}, {}
            for scan in scans:
                d = node_stage[scan]
                init = (
                    scan.init
                    if scan.init is not None
                    else ds._ACCUM_IDENTITY[scan.op]
                )
                seed[d] = orig_nas(init)
                if scan._subdim_step is not None:
                    step[d] = ds._Stage(AluOp.BYPASS, scan.expr)
            return seed, step

        def patched_nas(e):
            if isinstance(e, ds.Scan) and e._subdim_step is not None:
                return ds._Stage(e.op, ds.AluInp.CURR_ALU_OUT, e.expr)
            return orig_nas(e)

        uops_by_ver, shas = {}, {}
        ds._scan_overrides, ds._node_as_stage = patched_so, patched_nas
        try:
            for ver in ("v3", "v4"):
                uops_by_ver[ver] = lower(spec, ver=ver)
        finally:
            ds._scan_overrides, ds._node_as_stage = orig_so, orig_nas
        opcode = _CUSTOM_DVE_ROW_BASE + len(OPS)
        for ver in ("v3", "v4"):
            s = DveOpSpec(name=name, opcode=opcode, uops=uops_by_ver[ver], rd1_en=True)
            shas[ver] = s.sha(ver)
            _COMPILE_CACHE[(name, ver)] = s
        op = DveOp(name, spec, subdim=True, uops_sha=shas)
        _SUB_OPCODE_FOR_NAME[name] = opcode
        OPS.append(op)
        CUSTOM_DVE_SPECS[name] = spec
        return op

    def _uprime(src):
        return Bin(AluOp.BITWISE_OR, Bin(AluOp.BITWISE_AND, src, C0), C1)

    def _np_uprime(x):
        return (
            (x.view(np_.uint32) & PAYLOAD_MASK) | np_.uint32(0x4B000000)
        ).view(np_.float32)

    up_e, vp_e = _uprime(Src0), _uprime(Src1)
    wnum_spec = Spec(
        body=Bin(
            AluOp.MULTIPLY,
            ne(up_e, vp_e),
            Bin(AluOp.SUBTRACT, maxx(up_e, vp_e), C1),
        ),
        reference=lambda in0, in1, s0, s1, imm2: np_.where(
            _np_uprime(in0) != _np_uprime(in1),
            np_.maximum(_np_uprime(in0), _np_uprime(in1)) - np_.float32(F_2P23),
            np_.float32(0.0),
        ).astype(np_.float32),
    )
    spd_spec = Spec(
        body=Bin(
            AluOp.SUBTRACT, Bin(AluOp.ADD, up_e, vp_e), C2
        ),
        reference=lambda in0, in1, s0, s1, imm2: (
            _np_uprime(in0) + _np_uprime(in1) - np_.float32(F_2P24)
        ).astype(np_.float32),
    )

    embed = reg_embmax()
    wnum = reg("WNUM_ANT", wnum_spec, rd1=True)
    spd = reg("SPD_ANT", spd_spec, rd1=True)
    return embed, wnum, spd


TPB = 4                    # tiles per formula block
BW = TPB * R               # formula block width (1024)
NBLK = NT // TPB           # formula blocks per core


def _emit_tile(nc, pools, pred_v, targ_v, pay_b, ident_b, t, embed_op,
               mask_ap, me, mt, e2, s2a, s2b):
    """Streaming part for one [128, R*16] tile. Row stats land in column
    slice t%TPB of the block stats tiles me/mt; exp lands (w-major, so
    matmul rhs slices are contiguous) in half t%2 of the pair tile e2; on
    odd tiles 16 paired matmuls produce row sums of both halves into the
    alternating PSUM tiles s2a/s2b."""
    io_pool = pools[0]
    cols = slice((t % TPB) * R, (t % TPB + 1) * R)

    # both input streams on the sync HWDGE ring: SP issues nothing else, so
    # dma_starts go out back-to-back and are never stuck behind an ACT op
    xp = io_pool.tile([P, F], f32, tag="xp")
    nc.sync.dma_start(out=xp[:, :], in_=pred_v[t])
    xt = io_pool.tile([P, F], f32, tag="xt")
    nc.sync.dma_start(out=xt[:, :], in_=targ_v[t])

    # fused embed + segmented max over RAW predict on DVE; runs concurrently
    # with the exp pass on ACT (both only read xp)
    xp3 = xp[:, :].rearrange("p (r w) -> p r w", w=W)
    nc.vector._custom_dve(
        embed_op,
        out=me[:, cols].unsqueeze(2).broadcast_to([P, R, W]),
        in0=xp3, in1=pay_b, s0=mask_ap,
    )

    # E = exp(predict) on ScalarE into half t%2 of the bf16 pair tile
    # (contiguous write — a transposed write ran 5x slower on ACT); bf16 is
    # what lets TensorE stream it at 1 col/cycle
    nc.scalar.activation(e2[:, (t % 2) * F:(t % 2 + 1) * F], xp[:, :],
                         ACT.Exp)

    # target side: fused embed + segmented max on DVE
    xt3 = xt[:, :].rearrange("p (r w) -> p r w", w=W)
    nc.vector._custom_dve(
        embed_op,
        out=mt[:, cols].unsqueeze(2).broadcast_to([P, R, W]),
        in0=xt3, in1=pay_b, s0=mask_ap,
    )

    # row sums of E on the (otherwise idle) TensorE: 16 matmuls per tile
    # PAIR with identity weights, one per class column (strided rhs is fine
    # — matmul cost scales with out width only), PSUM-accumulated in f32
    # with even/odd w on separate banks so consecutive matmuls never
    # RMW-chain on the same bank; issued back-to-back so the PE stays at
    # its ramped p-state (~1ns/col vs ~2.1 when gappy):
    #   s2a+s2b = sum_w I.T @ E2[:, :, w]  ->  [128, 2R] each in PSUM
    if t % 2 == 1:
        e23 = e2[:, :].rearrange("p (hr w) -> p hr w", w=W)
        for w in range(W):
            dst = s2a if w % 2 == 0 else s2b
            nc.tensor.matmul(
                out=dst[:, :], lhsT=ident_b[:, :], rhs=e23[:, :, w],
                start=(w < 2), stop=(w >= W - 2),
            )


def _emit_f1(nc, pools, me, mt, s2c, ops, mask_ap, last):
    """Formula stage 1 for one [128, BW] block (emitted with its last
    tile): everything that only needs me/mt/s.
      wn = (u!=v)*max(u,v)   sp = u+v   em = exp(m)
      den = sp * sumexp      num = wn * em
    den/num run on GPSIMD (idle) except for the last block, where DVE's
    ~1.2us ops shorten the post-DMA tail. Returns (den, num)."""
    fp_pool = pools[3]
    _, wnum_op, spd_op = ops
    mul_eng = nc.vector if last else nc.gpsimd

    wn = fp_pool.tile([P, BW], f32, tag="wn")
    nc.vector._custom_dve(wnum_op, out=wn[:, :], in0=me[:, :], in1=mt[:, :],
                          s0=mask_ap, s1=F_2P23)
    sp = fp_pool.tile([P, BW], f32, tag="sp")
    nc.vector._custom_dve(spd_op, out=sp[:, :], in0=me[:, :], in1=mt[:, :],
                          s0=mask_ap, s1=F_2P23, imm2=F_2P24)
    # em = exp(m): payload bits perturb m by <= 2^-14 relative — in budget
    em = fp_pool.tile([P, BW], f32, tag="em")
    nc.scalar.activation(em[:, :], me[:, :], ACT.Exp)

    den = fp_pool.tile([P, BW], f32, tag="dn")
    mul_eng.tensor_tensor(den[:, :], sp[:, :], s2c[:, :], op=OP.mult)
    num = fp_pool.tile([P, BW], f32, tag="nm")
    mul_eng.tensor_tensor(num[:, :], wn[:, :], em[:, :], op=OP.mult)
    return den, num


def _emit_f2(nc, pools, res_sl, den, num, last):
    """Formula stage 2 (emitted one tile later so no engine head-of-line
    stalls on a cross-engine dep): res_block = num / den (host sums res)."""
    fp_pool = pools[3]
    rec = fp_pool.tile([P, BW], f32, tag="rc")
    nc.vector.reciprocal_approx_fast(out=rec[:, :], in_=den[:, :])
    mul_eng = nc.vector if last else nc.gpsimd
    mul_eng.tensor_tensor(res_sl, num[:, :], rec[:, :], op=OP.mult)


def _emit_pass(nc, pools, pred_v, targ_v, pay_b, ident_b, res, ops, mask_ap):
    _, work_pool, stats_pool, fp_pool, ps_pool = pools
    embed_op = ops[0]
    pend = None            # (den, num) of the previous block
    for k in range(NBLK):
        me = stats_pool.tile([P, BW], f32, tag="me")
        mt = stats_pool.tile([P, BW], f32, tag="mt")
        s2c = fp_pool.tile([P, BW], f32, tag="s2c")
        for half in range(TPB // 2):
            e2 = work_pool.tile([P, 2 * F], bf16, tag="e2")
            s2a = ps_pool.tile([P, 2 * R], f32, tag="s2a")
            s2b = ps_pool.tile([P, 2 * R], f32, tag="s2b")
            for sub in range(2):
                t = k * TPB + 2 * half + sub
                _emit_tile(nc, pools, pred_v, targ_v, pay_b, ident_b, t,
                           embed_op, mask_ap, me, mt, e2, s2a, s2b)
                # interleave the previous block's F2 early in this block
                if pend is not None and half == 0 and sub == 1:
                    _emit_f2(nc, pools, res[:, (k - 1) * BW:k * BW], *pend,
                             last=False)
                    pend = None
            # merge the two PSUM halves into the block s2c tile: only one
            # PSUM operand is allowed per instruction, so ACT drains s2a
            # to SBUF and DVE adds s2b on top
            sl = slice(half * 2 * R, (half + 1) * 2 * R)
            nc.scalar.activation(s2c[:, sl], s2a[:, :], ACT.Copy)
            nc.vector.tensor_tensor(s2c[:, sl], s2c[:, sl], s2b[:, :],
                                    op=OP.add)
        pend = _emit_f1(nc, pools, me, mt, s2c, ops, mask_ap,
                        last=(k == NBLK - 1))
    _emit_f2(nc, pools, res[:, (NBLK - 1) * BW:], *pend, last=True)


def _build_program():
    nc = bacc.Bacc("TRN2", target_bir_lowering=False, debug=False)
    pred = nc.dram_tensor("predict", [BS, W], f32, kind="ExternalInput")
    targ = nc.dram_tensor("target", [BS, W], f32, kind="ExternalInput")
    pay = nc.dram_tensor("payload", [P, W], u32, kind="ExternalInput")
    ident = nc.dram_tensor("ident", [P, P], f32, kind="ExternalInput")
    out = nc.dram_tensor("out", [P, NBLK * BW], f32, kind="ExternalOutput")

    pred_v = pred[:, :].rearrange("(t p r) w -> t p (r w)", t=NT, p=P, r=R)
    targ_v = targ[:, :].rearrange("(t p r) w -> t p (r w)", t=NT, p=P, r=R)

    with tile.TileContext(nc) as tc:
        with (
            tc.tile_pool(name="io", bufs=3) as io_pool,
            tc.tile_pool(name="work", bufs=2) as work_pool,
            tc.tile_pool(name="stats", bufs=2) as stats_pool,
            tc.tile_pool(name="fp", bufs=1) as fp_pool,
            tc.psum_pool(name="ps", bufs=3) as ps_pool,
            tc.tile_pool(name="const", bufs=1) as const_pool,
        ):
            pay_t = const_pool.tile([P, W], u32, tag="pay")
            nc.gpsimd.dma_start(out=pay_t[:, :], in_=pay[:, :])
            pay_b = pay_t[:, :].unsqueeze(1).broadcast_to([P, R, W]).bitcast(f32)

            ident_t = const_pool.tile([P, P], f32, tag="idf")
            nc.gpsimd.dma_start(out=ident_t[:, :], in_=ident[:, :])
            ident_b = const_pool.tile([P, P], bf16, tag="idb")
            nc.scalar.activation(ident_b[:, :], ident_t[:, :], ACT.Copy)

            mask_t = const_pool.tile([P, 1], u32, tag="mask")
            nc.vector.memset(mask_t[:, :], PAYLOAD_MASK)
            mask_ap = mask_t[:, :1].bitcast(f32)

            res = const_pool.tile([P, NBLK * BW], f32, tag="res")

            ops = _register_custom_ops()
            pools = (io_pool, work_pool, stats_pool, fp_pool, ps_pool)
            _emit_pass(nc, pools, pred_v, targ_v, pay_b, ident_b, res, ops,
                       mask_ap)

            nc.sync.dma_start(out=out[:, :], in_=res[:, :])
    nc.compile()
    return nc


_CACHE = {}


def _run(predict, target, trace=False):
    if "nc" not in _CACHE:
        _CACHE["nc"] = _build_program()
    nc = _CACHE["nc"]

    predict = np.ascontiguousarray(np.asarray(predict, dtype=np.float32))
    target = np.ascontiguousarray(np.asarray(target, dtype=np.float32))
    payload = np.broadcast_to(
        (np.asarray(LABELS_NUM_COUNT, dtype=np.uint32) // 1000)[None, :], (P, W)
    ).copy()
    ident = np.eye(P, dtype=np.float32)

    in_maps = []
    for i in range(NCORES):
        in_maps.append(
            {
                "predict": predict[i * BS : (i + 1) * BS],
                "target": target[i * BS : (i + 1) * BS],
                "payload": payload,
                "ident": ident,
            }
        )
    res = run_bass_kernel_spmd(nc, in_maps, core_ids=list(range(NCORES)), trace=trace)
    total = np.float64(0.0)
    for r in res.results:
        total += np.float64(r["out"].astype(np.float64).sum())
    value = np.float32(total / B)
    return np.asarray(value, dtype=np.float32), res


def kernel(predict, target, penalty_matrix=None):
    value, _ = _run(predict, target, trace=False)
    return value

